# revision 37
# baseline (speedup 1.0000x reference)
"""AttnDecoderRNN single-step decoder on 8 Trainium2 NeuronCores.

Model (batch=1): embedding row -> Bahdanau attention over 25 encoder states
-> combine linear + relu -> GRU cell -> vocab projection (V=50257)
-> log_softmax.  Returns (log_probs[1,V], h_new[1,1,H], attn_weights[1,ML]).

Everything is a matrix-vector product.  Weights stream through SBUF once and
are contracted against partition-replicated vectors with the fused DVE op
affine_mul_reduce (elementwise mult + free-dim sum per partition); biases are
added afterwards in one vector op per matrix.  Sharding (tensor-parallel over
output rows, per the vocab-sharding hint):

  - W_comb rows, GRU gate rows, and the vocab rows are split 8 ways.
  - Two 1KB AllGathers stitch the GRU input x and h_new back together; one
    256B AllGather shares per-core softmax sums so every core finishes
    log_softmax locally (streamed logsumexp with a fixed shift CV -- exact
    by shift invariance, no max pass needed at these logit magnitudes).
  - Vocab padded 50257 -> 51200 = 8*6400; pad rows get bias -1e30 so they
    vanish from the softmax; the host slices them off.

Per-core vocab projection (the memory-bound bulk) is further split across
engines: 3840 rows go through DVE affine_mul_reduce on natural-layout bf16
weights, 2560 rows go through the PE systolic array on host-pre-transposed
bf16 weights (16 PSUM-accumulated chunks; the bias rides in as a rank-1
matmul; pre-transposing on the host keeps the load at full DMA rate instead
of the ~206 GB/s xbar transpose path).  Weights are bf16 only for this
projection (halves the dominant traffic); activations, GRU, attention and
all other weights stay fp32.  Measured with hardware For_i differential
benches (loop_bench.py): fp32 DVE-only vocab phase 152.0us/core (= the
341 GB/s DMA roofline), bf16 DVE-only 117us (DVE 1x-bound), bf16 DVE+PE
split with device xbar transpose 97us, with host pre-transpose 79.3us
(back at the DMA roofline for the halved traffic).

The exp/tanh table set is pre-warmed at t=0 and sigmoid is computed as
0.5*(1+tanh(x/2)) so no ACT table switches land on the critical path; the
exp+ln set switch for the final log is hidden behind the last AllGather.
"""

import functools

import numpy as np

H = 2048
V = 50257
ML = 25
NC = 8
TPC = 50                 # vocab tiles per core (free-dim columns of logits)
VP = 128 * TPC           # 6400 vocab rows per core
VPAD = NC * VP           # 51200
SL = H // NC             # 256: per-core slice of H-sized outputs (comb, gates)
NEG = -1.0e30
WOUT_BF16 = True     # stream the vocab projection weights as bf16
PE_SPLIT = True      # vocab rows split: DVE amr half + PE transposed half
TPC_D = 30 if PE_SPLIT else TPC   # vocab tiles handled by the DVE half
VP_D = 128 * TPC_D               # rows in the DVE half
VP_P = VP - 0 if False else (VP - VP_D)   # rows in the PE half (mult of 512)
NCH = H // 128                   # 16 contraction chunks for the PE half


def _build_nc(single_core=False):
    import concourse.bacc as bacc
    import concourse.bass_isa as bass_isa
    import concourse.mybir as mybir
    import concourse.tile as tile

    f32 = mybir.dt.float32
    Alu = mybir.AluOpType
    Act = mybir.ActivationFunctionType
    RG = [list(range(NC))]

    nc = bacc.Bacc(
        "TRN2",
        target_bir_lowering=False,
        debug=False,
        enable_asserts=True,
        num_devices=1 if single_core else NC,
    )

    def all_gather(dst, src, nbytes_in):
        # dst = concat of all ranks' src. single_core: stand-in DMA for the
        # timeline model (values wrong, timing-ish right).
        if single_core:
            n = nbytes_in // 4
            srcv = src.rearrange("(a h) -> a h", a=1).broadcast_to((NC, n))
            nc.gpsimd.dma_start(dst.rearrange("(r h) -> r h", r=NC), srcv)
            return
        nc.gpsimd.collective_compute("AllGather", Alu.bypass,
                                     replica_groups=RG,
                                     ins=[src[:]], outs=[dst[:]])

    def din(name, shape):
        return nc.dram_tensor(name, shape, f32, kind="ExternalInput").ap()

    def dout(name, shape):
        return nc.dram_tensor(name, shape, f32, kind="ExternalOutput").ap()

    emb_row = din("emb_row", [1, H])
    h0 = din("h0", [1, H])
    h0c = din("h0c", [SL])
    enc = din("enc", [ML, H])
    W_attn = din("W_attn", [ML, 2 * H])
    b_attn = din("b_attn", [ML])
    W_comb = din("W_comb", [SL, 2 * H])
    b_comb = din("b_comb", [SL])
    W_ih = din("W_ih", [3 * SL, H])
    b_ih = din("b_ih", [3 * SL])
    W_hh = din("W_hh", [3 * SL, H])
    b_hh = din("b_hh", [3 * SL])
    wout_dt = mybir.dt.bfloat16 if WOUT_BF16 else f32
    if PE_SPLIT:
        W_out = nc.dram_tensor("W_out", [VP_D, H], wout_dt,
                               kind="ExternalInput").ap()
        b_out = din("b_out", [VP_D])
        W_out_p = nc.dram_tensor("W_out_p", [H, VP_P], wout_dt,
                                 kind="ExternalInput").ap()
        b_out_p = nc.dram_tensor("b_out_p", [VP_P], mybir.dt.bfloat16,
                                 kind="ExternalInput").ap()
    else:
        W_out = nc.dram_tensor("W_out", [VP, H], wout_dt,
                               kind="ExternalInput").ap()
        b_out = din("b_out", [VP])

    out_logp = dout("out_logp", [VP_D if PE_SPLIT else VP])
    if PE_SPLIT:
        out_logp_p = dout("out_logp_p", [VP_P])
    out_hnew = dout("out_hnew", [SL])
    out_attnw = dout("out_attnw", [ML])

    def amr(wtile, xrep, acc):
        # acc = sum(wtile * xrep) per partition (seed=0); wtile clobbered.
        nc.vector.affine_mul_reduce(
            out=wtile,
            accum_out=acc,
            in0=wtile,
            in1=xrep,
            scale=1.0,
            bias=0.0,
        )

    CV = 20.0   # fixed logsumexp shift for vocab logits (|logit| << 20)
    CA = 60.0   # fixed softmax shift for attention scores (|score| << 60)

    with tile.TileContext(nc) as tc:
        with tc.tile_pool(name="consts", bufs=1) as consts, \
             tc.tile_pool(name="wstream", bufs=6) as wpool, \
             tc.tile_pool(name="work", bufs=1) as work, \
             tc.tile_pool(name="pp", bufs=2, space="PSUM") as pp, \
             tc.tile_pool(name="dram", bufs=1, space="DRAM") as dram:

            ones = consts.tile([1, 128], f32)
            nc.vector.memset(ones[:], 1.0)
            ones_col = consts.tile([128, 1], f32)
            nc.vector.memset(ones_col[:], 1.0)
            warm = consts.tile([1, 1], f32)
            nc.scalar.activation(warm[:], ones[:, 0:1], Act.Exp)

            def replicate(dst, src_sb, channels=128):
                # dst[C, N] = src_sb[1, N] on every partition via GpSimd
                # (engine otherwise idle; 8KB staged once, no 128x HBM
                # re-read like a stride-0 broadcast DMA would do).
                nc.gpsimd.partition_broadcast(dst, src_sb, channels=channels)

            def pe_apply(dst, lhsT, rhs_sb, nk):
                # dst[128, N] (SBUF) = lhsT.T @ rhs_sb via PE, staged
                # through a 2-bank PSUM tile in 1024-wide passes.
                n = dst.shape[-1]
                for j0 in range(0, n, 1024):
                    ps = pp.tile([128, 1024], f32, tag="rep", bufs=1,
                                 name=f"ps{nc.next_id()}")
                    wdt_ = min(1024, n - j0)
                    for j in range(0, wdt_, 512):
                        w = min(512, wdt_ - j)
                        nc.tensor.matmul(ps[:, j:j + w], lhsT=lhsT,
                                         rhs=rhs_sb[:, j0 + j:j0 + j + w],
                                         start=True, stop=True)
                    nc.scalar.copy(dst[:, j0:j0 + wdt_], ps[:, 0:wdt_])

            # small vectors up front (scalar-ring DMAs, off the weight stream)
            emb_sb = work.tile([1, H], f32, tag="vec", bufs=2)
            nc.scalar.dma_start(emb_sb[:], emb_row[:])
            h0_sb = work.tile([1, H], f32, tag="vec", bufs=2)
            nc.scalar.dma_start(h0_sb[:], h0[:])
            bat = consts.tile([ML, 1], f32)
            nc.scalar.dma_start(bat[:], b_attn.rearrange("(p u) -> p u", u=1))
            bcomb = consts.tile([128, 2], f32)
            nc.scalar.dma_start(bcomb[:], b_comb.rearrange("(p u) -> p u", u=2))
            bih = consts.tile([128, 3, 2], f32)
            nc.scalar.dma_start(bih[:], b_ih.rearrange("(g p u) -> p g u",
                                                       g=3, p=128, u=2))
            bhh = consts.tile([128, 3, 2], f32)
            nc.scalar.dma_start(bhh[:], b_hh.rearrange("(g p u) -> p g u",
                                                       g=3, p=128, u=2))
            h0cs = work.tile([128, 2], f32)
            nc.scalar.dma_start(h0cs[:], h0c.rearrange("(p u) -> p u", u=2))
            bout = consts.tile([128, TPC_D], f32)
            nc.scalar.dma_start(bout[:],
                                b_out.rearrange("(p t) -> p t", t=TPC_D))
            if PE_SPLIT:
                boutp = consts.tile([1, VP_P], mybir.dt.bfloat16)
                nc.scalar.dma_start(
                    boutp[:], b_out_p.rearrange("(a v) -> a v", a=1))
                onesb = consts.tile([1, 1], mybir.dt.bfloat16)
                nc.vector.memset(onesb[:], 1.0)

            # h0 replicated: needed by the W_hh matvecs right away
            h0_rep = work.tile([128, H], f32)
            replicate(h0_rep[:], h0_sb[:])

            # ---------------- attention (identical on every core) ----------
            emb_rep = work.tile([128, H], f32)
            replicate(emb_rep[:], emb_sb[:])

            wat = wpool.tile([ML, 2 * H], f32, tag="w4", bufs=2, name="wat")
            nc.sync.dma_start(wat[:], W_attn[:])
            enc_sb = consts.tile([ML, H], f32)
            nc.sync.dma_start(enc_sb[:], enc[:])

            # softmax over 25 scores on the partition axis, padded to 32
            # (partition_all_reduce needs channels % 32 == 0).  Fixed shift
            # CA stands in for the max -- softmax is shift invariant and
            # scores are tiny, so exp(s - CA) cannot over/underflow; the
            # -1e30 pads contribute exp() = 0.
            scores = work.tile([32, 1], f32)
            sc2 = work.tile([32, 1], f32)
            nc.vector.memset(scores[:], NEG)
            nc.vector.memset(sc2[:], 0.0)
            amr(wat[:, 0:H], emb_rep[0:ML, :], scores[0:ML, :])
            amr(wat[:, H:2 * H], h0_rep[0:ML, :], sc2[0:ML, :])
            nc.vector.tensor_add(scores[0:ML, :], scores[0:ML, :],
                                 sc2[0:ML, :])
            nc.vector.tensor_add(scores[0:ML, :], scores[0:ML, :], bat[:])
            negca = consts.tile([32, 1], f32)
            nc.vector.memset(negca[:], -CA)
            ex = work.tile([32, 1], f32)
            nc.scalar.activation(ex[:], scores[:], Act.Exp, bias=negca[:])
            sm = work.tile([32, 1], f32)
            nc.gpsimd.partition_all_reduce(sm[:], ex[:], channels=32,
                                           reduce_op=bass_isa.ReduceOp.add)
            rs = work.tile([32, 1], f32)
            nc.vector.reciprocal(rs[:], sm[:])
            aw = work.tile([32, 1], f32)
            nc.vector.tensor_mul(aw[:], ex[:], rs[:])
            nc.scalar.dma_start(out_attnw.rearrange("(p u) -> p u", u=1),
                                aw[0:ML, :])

            # ---------------- GRU gate-h matvecs (depend only on h0) -------
            gh = work.tile([128, 6], f32)
            whhv = W_hh.rearrange("(g p u) h -> g p u h", g=3, p=128, u=2)
            for g in range(3):
                for u in range(2):
                    wh = wpool.tile([128, H], f32, tag="w2", bufs=8,
                                    name=f"whh{g}{u}")
                    nc.sync.dma_start(wh[:], whhv[g, :, u, :])
                    amr(wh[:], h0_rep[:], gh[:, 2 * g + u:2 * g + u + 1])
            nc.vector.tensor_add(gh[:], gh[:],
                                 bhh.rearrange("p a b -> p (a b)"))

            # attn_applied, replicated to all partitions in one shot:
            # lhsT = aw broadcast to [25, 128] so every output partition
            # computes the same attention-weighted sum.
            awrep = work.tile([ML, 128], f32)
            nc.vector.memset(awrep[:], 1.0)
            nc.scalar.activation(awrep[:], awrep[:], Act.Copy,
                                 scale=aw[0:ML, 0:1])
            app_sb = work.tile([128, H], f32)
            pe_apply(app_sb[:], awrep[:], enc_sb[:], ML)

            # ---------------- combine linear (sharded rows) -----------------
            yx = work.tile([128, 2], f32)
            yx2 = work.tile([128, 2], f32)
            wcv = W_comb.rearrange("(p u) d -> u p d", p=128, u=2)
            for u in range(2):
                wc = wpool.tile([128, 2 * H], f32, tag="w4", bufs=2, name=f"wc{u}")
                nc.sync.dma_start(wc[:], wcv[u])
                amr(wc[:, 0:H], emb_rep[:], yx[:, u:u + 1])
                amr(wc[:, H:2 * H], app_sb[:], yx2[:, u:u + 1])
            nc.vector.tensor_add(yx[:], yx[:], yx2[:])
            nc.vector.tensor_add(yx[:], yx[:], bcomb[:])
            xs = work.tile([128, 2], f32)
            nc.vector.tensor_scalar_max(xs[:], yx[:], 0.0)

            ccx_in = dram.tile([SL], f32)
            nc.scalar.dma_start(ccx_in.rearrange("(p u) -> p u", u=2), xs[:])
            ccx_out = dram.tile([H], f32, addr_space="Shared")
            all_gather(ccx_out, ccx_in, SL * 4)
            x_rep = work.tile([128, H], f32)
            nc.scalar.dma_start(
                x_rep[:],
                ccx_out.rearrange("(a h) -> a h", a=1).broadcast_to((128, H)))

            # ---------------- GRU gate-i matvecs + cell ---------------------
            gi = work.tile([128, 6], f32)
            wihv = W_ih.rearrange("(g p u) h -> g p u h", g=3, p=128, u=2)
            for g in range(3):
                for u in range(2):
                    wi = wpool.tile([128, H], f32, tag="w2", bufs=8,
                                    name=f"wih{g}{u}")
                    nc.sync.dma_start(wi[:], wihv[g, :, u, :])
                    amr(wi[:], x_rep[:], gi[:, 2 * g + u:2 * g + u + 1])
            nc.vector.tensor_add(gi[:], gi[:],
                                 bih.rearrange("p a b -> p (a b)"))

            rt = work.tile([128, 2], f32)
            nc.vector.tensor_add(rt[:], gi[:, 0:2], gh[:, 0:2])
            r = work.tile([128, 2], f32)
            nc.scalar.activation(r[:], rt[:], Act.Tanh, scale=0.5)
            nc.vector.tensor_scalar(r[:], r[:], 0.5, 0.5,
                                    op0=Alu.mult, op1=Alu.add)
            zt = work.tile([128, 2], f32)
            nc.vector.tensor_add(zt[:], gi[:, 2:4], gh[:, 2:4])
            z = work.tile([128, 2], f32)
            nc.scalar.activation(z[:], zt[:], Act.Tanh, scale=0.5)
            nc.vector.tensor_scalar(z[:], z[:], 0.5, 0.5,
                                    op0=Alu.mult, op1=Alu.add)
            nt = work.tile([128, 2], f32)
            nc.vector.tensor_mul(nt[:], r[:], gh[:, 4:6])
            nc.vector.tensor_add(nt[:], nt[:], gi[:, 4:6])
            n = work.tile([128, 2], f32)
            nc.scalar.activation(n[:], nt[:], Act.Tanh)

            hn = work.tile([128, 2], f32)
            nc.vector.tensor_sub(hn[:], h0cs[:], n[:])
            nc.vector.tensor_mul(hn[:], hn[:], z[:])
            nc.vector.tensor_add(hn[:], hn[:], n[:])
            nc.scalar.dma_start(out_hnew.rearrange("(p u) -> p u", u=2), hn[:])

            cch_in = dram.tile([SL], f32)
            nc.scalar.dma_start(cch_in.rearrange("(p u) -> p u", u=2), hn[:])
            cch_out = dram.tile([H], f32, addr_space="Shared")
            all_gather(cch_out, cch_in, SL * 4)
            hn_rep = work.tile([128, H], f32)
            nc.scalar.dma_start(
                hn_rep[:],
                cch_out.rearrange("(a h) -> a h", a=1).broadcast_to((128, H)))

            # ---------------- vocab projection (sharded) --------------------
            if PE_SPLIT:
                # bf16 copy of h_new laid out [p, c] = hn[c*128 + p] for
                # the PE half's per-chunk stationary vectors
                hnT = work.tile([128, NCH], f32)
                nc.scalar.dma_start(
                    hnT[:], cch_out.rearrange("(c p) -> p c", c=NCH, p=128))
                hnTb = work.tile([128, NCH], mybir.dt.bfloat16)
                nc.vector.tensor_copy(hnTb[:], hnT[:])

            lg = work.tile([128, TPC_D], f32)
            wov = W_out.rearrange("(p t) h -> p t h", p=128, t=TPC_D)
            if WOUT_BF16:
                for t0 in range(0, TPC_D, 2):
                    wo = wpool.tile([128, 2, H], wout_dt, tag="w2", bufs=8,
                                    name=f"wo{t0}")
                    nc.sync.dma_start(wo[:], wov[:, t0:t0 + 2, :])
                    for j in range(2):
                        amr(wo[:, j, :], hn_rep[:], lg[:, t0 + j:t0 + j + 1])
            else:
                for t in range(TPC_D):
                    wo = wpool.tile([128, H], f32, tag="w2", bufs=8,
                                    name=f"wo{t}")
                    nc.sync.dma_start(wo[:], wov[:, t, :])
                    amr(wo[:], hn_rep[:], lg[:, t:t + 1])
            nc.vector.tensor_add(lg[:], lg[:], bout[:])

            if PE_SPLIT:
                # PE half: logits_p[v] = sum_c hn_chunk_c . W_p.T chunk_c[v]
                # accumulated over NCH chunks in PSUM; bias folded in as a
                # final rank-1 matmul against onesb.
                nvg = VP_P // 512
                lgp = work.tile([1, VP_P], f32)
                pps = [pp.tile([1, 512], f32, tag=f"pev{vg}", bufs=1,
                               name=f"pev{vg}") for vg in range(nvg)]
                for c in range(NCH):
                    wt = wpool.tile([128, VP_P], mybir.dt.bfloat16,
                                    tag="wt", bufs=4, name=f"wt{c}")
                    nc.sync.dma_start(wt[:],
                                      W_out_p[c * 128:(c + 1) * 128, :])
                    for vg in range(nvg):
                        nc.tensor.matmul(
                            pps[vg][0:1, :], lhsT=hnTb[:, c:c + 1],
                            rhs=wt[:, vg * 512:(vg + 1) * 512],
                            start=(c == 0), stop=False)
                for vg in range(nvg):
                    nc.tensor.matmul(
                        pps[vg][0:1, :], lhsT=onesb[:],
                        rhs=boutp[:, vg * 512:(vg + 1) * 512],
                        start=False, stop=True)
                    nc.scalar.copy(lgp[:, vg * 512:(vg + 1) * 512],
                                   pps[vg][0:1, :])

            # ---------------- streamed log_softmax --------------------------
            # Z = sum exp(lg - CV); fixed CV replaces the max pass (shift
            # invariance; logits are O(1) so no over/underflow at CV=20).
            negcv = consts.tile([128, 1], f32)
            nc.vector.memset(negcv[:], -CV)
            es = work.tile([128, TPC_D], f32)
            srow = work.tile([128, 1], f32)
            nc.scalar.activation(es[:], lg[:], Act.Exp, bias=negcv[:],
                                 accum_out=srow[:])
            score_s = work.tile([128, 1], f32)
            nc.gpsimd.partition_all_reduce(score_s[:], srow[:], channels=128,
                                           reduce_op=bass_isa.ReduceOp.add)
            if PE_SPLIT:
                esp = work.tile([1, VP_P], f32, tag="pescr", bufs=1)
                sp = work.tile([1, 1], f32)
                nc.scalar.activation(esp[:], lgp[:], Act.Exp,
                                     bias=negcv[0:1, :], accum_out=sp[:])
            warmln = work.tile([1, 1], f32)
            nc.scalar.activation(warmln[:], ones[:, 0:1], Act.Ln)

            pk = work.tile([1, 8], f32)
            nc.vector.memset(pk[:], 0.0)
            if PE_SPLIT:
                nc.vector.tensor_add(pk[:, 0:1], score_s[0:1, 0:1], sp[:])
            else:
                nc.vector.tensor_copy(pk[:, 0:1], score_s[0:1, 0:1])
            ccms_in = dram.tile([8], f32)
            nc.scalar.dma_start(ccms_in.rearrange("(a k) -> a k", a=1), pk[:])
            ccms_out = dram.tile([8 * NC], f32, addr_space="Shared")
            all_gather(ccms_out, ccms_in, 8 * 4)
            mss = work.tile([128, NC], f32)
            nc.scalar.dma_start(
                mss[:],
                ccms_out.rearrange("(r k) -> k r", k=8)[0:1, :]
                .broadcast_to((128, NC)))

            ds = work.tile([128, 1], f32)
            nc.vector.reduce_sum(ds[:], mss[:], axis=mybir.AxisListType.X)
            ld = work.tile([128, 1], f32)
            nc.scalar.activation(ld[:], ds[:], Act.Ln)
            nlz_rep = work.tile([128, 1], f32)
            nc.vector.tensor_scalar(nlz_rep[:], ld[:], CV, -1.0,
                                    op0=Alu.add, op1=Alu.mult)

            logp = work.tile([128, TPC_D], f32)
            nc.vector.tensor_scalar_add(logp[:], lg[:], nlz_rep[:])
            nc.scalar.dma_start(out_logp.rearrange("(p t) -> p t", t=TPC_D),
                                logp[:])
            if PE_SPLIT:
                logpp = work.tile([1, VP_P], f32, tag="pescr", bufs=1)
                nc.vector.tensor_scalar_add(logpp[:], lgp[:],
                                            nlz_rep[0:1, :])
                nc.scalar.dma_start(
                    out_logp_p.rearrange("(a v) -> a v", a=1), logpp[:])

    nc.compile()
    return nc


@functools.lru_cache(maxsize=1)
def _get_nc():
    return _build_nc()


def _per_core_inputs(inputs):
    import ml_dtypes
    wdt = ml_dtypes.bfloat16 if WOUT_BF16 else np.float32
    f = np.float32
    emb = np.asarray(inputs["emb"], f)
    tok = int(np.asarray(inputs["input_tok"]).reshape(-1)[0])
    emb_row = np.ascontiguousarray(emb[tok:tok + 1, :])
    h0 = np.asarray(inputs["hidden"], f).reshape(1, H)
    enc = np.asarray(inputs["encoder_outputs"], f)
    W_attn = np.asarray(inputs["W_attn"], f)
    b_attn = np.asarray(inputs["b_attn"], f)
    W_comb = np.asarray(inputs["W_comb"], f)
    b_comb = np.asarray(inputs["b_comb"], f)
    W_ih = np.asarray(inputs["W_ih"], f)
    b_ih = np.asarray(inputs["b_ih"], f)
    W_hh = np.asarray(inputs["W_hh"], f)
    b_hh = np.asarray(inputs["b_hh"], f)
    W_out = np.asarray(inputs["W_out"], f)
    b_out = np.asarray(inputs["b_out"], f)

    in_maps = []
    for c in range(NC):
        s = slice(c * SL, (c + 1) * SL)
        gsl = [slice(g * H + c * SL, g * H + (c + 1) * SL) for g in range(3)]
        v0, v1 = c * VP, (c + 1) * VP
        if v1 <= V:
            woc = W_out[v0:v1].astype(wdt)
            boc = b_out[v0:v1].astype(f)
        else:
            nreal = max(V - v0, 0)
            woc = np.zeros((VP, H), wdt)
            woc[:nreal] = W_out[v0:V].astype(wdt)
            boc = np.full((VP,), NEG, f)
            boc[:nreal] = b_out[v0:V]
        extra = {}
        if PE_SPLIT:
            import ml_dtypes
            extra = {
                "W_out_p": np.ascontiguousarray(woc[VP_D:].T),
                "b_out_p": np.ascontiguousarray(
                    boc[VP_D:]).astype(ml_dtypes.bfloat16),
            }
            woc = woc[:VP_D]
            boc = boc[:VP_D]
        in_maps.append({**extra,
            "emb_row": emb_row,
            "h0": h0,
            "h0c": np.ascontiguousarray(h0[0, s]),
            "enc": enc,
            "W_attn": W_attn,
            "b_attn": b_attn,
            "W_comb": np.ascontiguousarray(W_comb[s]),
            "b_comb": np.ascontiguousarray(b_comb[s]),
            "W_ih": np.ascontiguousarray(np.concatenate([W_ih[g] for g in gsl])),
            "b_ih": np.ascontiguousarray(np.concatenate([b_ih[g] for g in gsl])),
            "W_hh": np.ascontiguousarray(np.concatenate([W_hh[g] for g in gsl])),
            "b_hh": np.ascontiguousarray(np.concatenate([b_hh[g] for g in gsl])),
            "W_out": np.ascontiguousarray(woc),
            "b_out": np.ascontiguousarray(boc),
        })
    return in_maps


def _assemble(results):
    if PE_SPLIT:
        logp = np.concatenate(
            [np.concatenate([results[c]["out_logp"],
                             results[c]["out_logp_p"]])
             for c in range(NC)])[:V]
    else:
        logp = np.concatenate(
            [results[c]["out_logp"] for c in range(NC)])[:V]
    hnew = np.concatenate([results[c]["out_hnew"] for c in range(NC)])
    attnw = results[0]["out_attnw"]
    return (logp.reshape(1, V).astype(np.float32),
            hnew.reshape(1, 1, H).astype(np.float32),
            attnw.reshape(1, ML).astype(np.float32))


def kernel(**inputs):
    from concourse import bass_utils
    nc = _get_nc()
    in_maps = _per_core_inputs(inputs)
    res = bass_utils.run_bass_kernel_spmd(
        nc, in_maps, core_ids=list(range(NC)), trace=False)
    return _assemble(res.results)


if __name__ == "__main__":
    rng = np.random.default_rng(0)
    fake = {
        "input_tok": np.array([123], np.int64),
        "hidden": rng.standard_normal((1, 1, H), dtype=np.float32),
        "encoder_output": rng.standard_normal((1, H), dtype=np.float32),
        "encoder_outputs": rng.standard_normal((ML, H), dtype=np.float32),
        "emb": (rng.standard_normal((V, H), dtype=np.float32) * 0.02),
        "W_attn": (rng.standard_normal((ML, 2 * H), dtype=np.float32) * 0.02),
        "b_attn": np.zeros((ML,), np.float32),
        "W_comb": (rng.standard_normal((H, 2 * H), dtype=np.float32) * 0.02),
        "b_comb": np.zeros((H,), np.float32),
        "W_ih": (rng.standard_normal((3 * H, H), dtype=np.float32) * 0.02),
        "b_ih": np.zeros((3 * H,), np.float32),
        "W_hh": (rng.standard_normal((3 * H, H), dtype=np.float32) * 0.02),
        "b_hh": np.zeros((3 * H,), np.float32),
        "W_out": (rng.standard_normal((V, H), dtype=np.float32) * 0.02),
        "b_out": np.zeros((V,), np.float32),
    }
    outs = kernel(**fake)
    for o in outs:
        print(o.shape, o.dtype, float(np.abs(o).max()))
